# revision 5
# baseline (speedup 1.0000x reference)
"""3-layer GCN stack (PyG GCNConv semantics w/ self-loops + symmetric norm,
relu, global mean pool) on 8 Trainium2 NeuronCores via Bass/Tile.

Distribution: nodes sharded graph-contiguously across 8 cores (pooling stays
local); per layer each core transforms its shard t = dis*(h@W) in fp16 and the
shards are AllGathered into a replicated message table; each core then
processes its own dst nodes in 127-node windows: per-edge rows are fetched
with gpsimd dma_gather (int16 idxs -> <=4 src-block passes of 32768 table
rows) and segment-summed on the PE via suffix-staircase matmuls accumulated in
fp32 PSUM; relu runs on ACT.  Global mean pool is a final per-window matmul
against a host-built dis-weighted selection matrix accumulated in a
persistent PSUM tile.

v2: bias==0 lets the dst-side deg-scale fold into the next layer's
transform (scale dis^2) and into the pooling matrix, removing the disb
matmul and the DVE multiply from every window.  The table is split into 4
window-chunk Shared sub-tables (each one int16 gather view), AllGathered
independently as their windows flush, so collectives pipeline with compute
and next-layer gathers.  Segments pack without rounding to tiles: a tile
straddling two windows is accumulated into both, and because the staircase
sentinel column is the per-core true segment end, foreign-window edges and
arbitrary-content pad rows cancel exactly in the suffix differences — no
zero-row machinery, ~11%% fewer gathered rows.

All per-core streams share one compile-time template (max over cores).
"""
import sys

if "/opt/trn_rl_repo" not in sys.path:
    sys.path.insert(0, "/opt/trn_rl_repo")

import numpy as np

import concourse.bacc as bacc
import concourse.bass as bass
import concourse.mybir as mybir
import concourse.tile as tile
from concourse.bass_utils import run_bass_kernel_spmd
from concourse.library_config import mlp as _mlp_lib

NCORES = 8
WIN = 127            # dst nodes per window (col 127 is the suffix sentinel)
BLOCK = 32768        # dma_gather int16 index range per table view
CHUNK = 2048         # gather call granularity (indices)
G_SLOTS = 16         # pooling graph slots per core
N_QUEUES = 2         # swdge queues for dma_gather
MP_BUFS = 6          # gather destination double-buffer depth (per block tag)
CS_BUFS = 12         # cs selection-matrix lookahead tiles
XCHUNK = 4           # windows per transposed x load (prologue)
F16 = mybir.dt.float16
F32 = mybir.dt.float32
I16 = mybir.dt.int16


# --------------------------------------------------------------------------
# host-side preprocessing
# --------------------------------------------------------------------------

def _build_host(x, edge_index, batch):
    N, H = x.shape
    G = int(batch.max()) + 1 if batch.size else 1
    src = np.asarray(edge_index[0], dtype=np.int64)
    dst = np.asarray(edge_index[1], dtype=np.int64)
    batch = np.asarray(batch, dtype=np.int64)

    deg = np.bincount(dst, minlength=N).astype(np.float64) + 1.0
    dis = (1.0 / np.sqrt(deg)).astype(np.float32)

    # --- partition graphs -> cores (contiguous node ranges) ---
    gsizes = np.bincount(batch, minlength=G)
    gends = np.cumsum(gsizes)                       # node index after graph g
    cuts = [0]
    for c in range(1, NCORES):
        target = round(N * c / NCORES)
        gi = min(int(np.searchsorted(gends, target)), G - 1)
        lo = int(gends[gi - 1]) if gi > 0 else 0
        hi = int(gends[gi])
        cut = lo if abs(lo - target) <= abs(hi - target) else hi
        cuts.append(max(cut, cuts[-1]))
    cuts.append(N)
    n0 = np.array(cuts[:-1], dtype=np.int64)
    n1 = np.array(cuts[1:], dtype=np.int64)
    counts = n1 - n0
    n_win = int(np.ceil(counts.max() / WIN)) + 1     # +1 all-ghost window
    S_pad = n_win * WIN                              # shard rows (ghosts incl)

    # --- window groups (processing granularity) and window->chunk split ---
    # chunks are contiguous group ranges of <=32 windows; the table is split
    # into one Shared sub-table per chunk (8*32*127 <= 32768 rows, a single
    # int16 dma_gather view), AllGathered independently for pipelining.
    # (wgroups built on tile counts later must align; groups here are the
    # fixed 8-window tiling, final groups may be smaller.)
    MAXW = 32
    chunk_w0 = [0]
    while chunk_w0[-1] + MAXW < n_win:
        chunk_w0.append(chunk_w0[-1] + MAXW)
    chunk_w0.append(n_win)
    NBLK = len(chunk_w0) - 1
    assert NBLK <= 4
    substep = [(chunk_w0[k + 1] - chunk_w0[k]) * WIN for k in range(NBLK)]
    sub_rows = [NCORES * s for s in substep]
    assert all(r <= BLOCK for r in sub_rows)
    chunk_of_w = np.zeros(n_win, dtype=np.int64)
    for k in range(NBLK):
        chunk_of_w[chunk_w0[k]:chunk_w0[k + 1]] = k
    TBL_ROWS = NCORES * S_pad

    core_of = np.searchsorted(n1 - 1, np.arange(N), side="left")
    loc = np.arange(N) - n0[core_of]
    lw = loc // WIN                                  # local window
    kk = chunk_of_w[lw]                              # chunk of node
    substep_a = np.array(substep, dtype=np.int64)
    cw0_rows = np.array([chunk_w0[k] * WIN for k in range(NBLK)],
                        dtype=np.int64)
    # row within sub-table kk
    rowmap = core_of * substep_a[kk] + (loc - cw0_rows[kk])

    # --- per-core edge streams (edges + self loops, owned by dst core) ---
    all_src = np.concatenate([src, np.arange(N, dtype=np.int64)])
    all_dst = np.concatenate([dst, np.arange(N, dtype=np.int64)])
    e_core = core_of[all_dst]
    e_row = rowmap[all_src]
    e_dloc = all_dst - n0[e_core]
    e_win = e_dloc // WIN
    e_blk = kk[all_src]                              # chunk/view of src

    # segment lengths per (core, win, blk) -> shared tile template.
    # segments are packed WITHOUT rounding to tiles: a 128-edge tile may
    # straddle two windows; it is accumulated into both, and the staircase
    # differences (with sentinel = per-core true end) cancel the foreign
    # edges exactly.
    seg_id = (e_core * n_win + e_win) * NBLK + e_blk
    seg_len = np.bincount(seg_id, minlength=NCORES * n_win * NBLK)
    seg_len = seg_len.reshape(NCORES, n_win, NBLK)
    seg_cap = seg_len.max(axis=0)                    # [n_win, NBLK] no ceil
    # block-major template: region per block, windows packed inside, region
    # end padded to a whole tile
    seg_off3 = np.zeros((n_win, NBLK), dtype=np.int64)
    off = 0
    region = np.zeros((NBLK, 2), dtype=np.int64)
    for b in range(NBLK):
        region[b, 0] = off
        for w in range(n_win):
            seg_off3[w, b] = off
            off += int(seg_cap[w, b])
        off = (off + 127) // 128 * 128
        region[b, 1] = off
    TOT_IDX = int(off)
    TOT_TILES = TOT_IDX // 128

    # pad positions point at view row 0: their (arbitrary) values cancel in
    # the staircase differences since the sentinel column uses the per-core
    # true segment end.
    idx_stream = np.zeros((NCORES, TOT_IDX), dtype=np.int16)

    # sort edges by (core, win, blk, dloc); scatter into template positions
    order = np.lexsort((e_dloc, e_blk, e_win, e_core))
    s_core = e_core[order]
    s_win = e_win[order]
    s_blk = e_blk[order]
    s_dloc = e_dloc[order]
    s_idx16 = e_row[order].astype(np.int16)          # row within sub-table
    s_seg = (s_core * n_win + s_win) * NBLK + s_blk
    seg_first = np.concatenate([[True], s_seg[1:] != s_seg[:-1]])
    first_pos = np.flatnonzero(seg_first)
    run_id = np.cumsum(seg_first) - 1
    rank = np.arange(s_seg.size) - first_pos[run_id]
    pos = seg_off3[s_win, s_blk] + rank
    idx_stream[s_core, pos] = s_idx16

    # starts: per core, per tile, 128 cols (127 nodes + sentinel), clamped
    node_in_seg = s_seg * WIN + (s_dloc % WIN)
    cnt = np.bincount(node_in_seg, minlength=NCORES * n_win * NBLK * WIN)
    cnt = cnt.reshape(NCORES, n_win, NBLK, WIN)
    starts_col = np.concatenate(
        [np.zeros((NCORES, n_win, NBLK, 1), np.int64),
         np.cumsum(cnt, axis=3)], axis=3)            # [.., WIN+1]
    # per-window tile instances: (w, b) touches template tiles
    # [off//128, ceil((off+cap)/128)); entry = (tile position p, beta)
    wtinst = {w: [] for w in range(n_win)}           # w -> [(p, w, b, beta)]
    for w in range(n_win):
        for b in range(NBLK):
            o, cap = int(seg_off3[w, b]), int(seg_cap[w, b])
            if cap == 0:
                continue
            for tt in range(o // 128, (o + cap + 127) // 128):
                wtinst[w].append((tt * 128, b, tt * 128 - o))

    # group-packed starts layout: windows grouped by starts budget; within
    # a group, tile instances ordered (window, blk, tile).
    wgroups = []
    cur, cols = [], 0
    for w in range(n_win):
        ntw = len(wtinst[w])
        if cur and (cols + ntw * 128 > 8192 or len(cur) >= 8
                    or w in chunk_w0):
            wgroups.append(cur)
            cur, cols = [], 0
        cur.append(w)
        cols += ntw * 128
    if cur:
        wgroups.append(cur)
    gtiles = []     # per group: list of (w, p) in packed order
    gbase = []      # per group: packed base col
    pk = 0
    pk_of = {}      # (w, p) -> packed col
    for grp in wgroups:
        gbase.append(pk)
        tl = []
        for w in grp:
            for p, b, beta in wtinst[w]:
                tl.append((w, p))
                pk_of[(w, p)] = pk
                pk += 128
        gtiles.append(tl)
        pk = (pk + 511) // 512 * 512          # align next group base
    PK_TOT = pk
    starts_packed = np.zeros((NCORES, PK_TOT // 128, 128), dtype=np.float16)
    for w in range(n_win):
        for p, b, beta in wtinst[w]:
            # col WIN (sentinel) = per-core true segment end, so trailing
            # pads and foreign-window edges cancel in the differences
            v = starts_col[:, w, b, :] - beta            # [NCORES, WIN+1]
            starts_packed[:, pk_of[(w, p)] // 128, :] = \
                np.clip(v, -2, 130).astype(np.float16)

    # wrapped idx layout [128, TOT_IDX/16]
    idx_wrapped = idx_stream.reshape(NCORES, TOT_IDX // 16, 16).transpose(0, 2, 1)
    idx_wrapped = np.tile(idx_wrapped, (1, 8, 1))

    # dis arrays (window rows padded to 128)
    dis_pad = np.zeros((NCORES, n_win, 128), dtype=np.float32)
    for c in range(NCORES):
        dv = dis[n0[c]:n1[c]]
        loc = np.arange(counts[c])
        dis_pad[c, loc // WIN, loc % WIN] = dv

    # pooling CS (dis-weighted: dst-side deg scale is folded in here for the
    # last layer) + recip
    first_graph = batch[np.minimum(n0, N - 1)]
    cs_pool = np.zeros((NCORES, n_win * 128, G_SLOTS), dtype=np.float16)
    recip = np.zeros((NCORES, G_SLOTS, 1), dtype=np.float32)
    gcount = np.bincount(batch, minlength=G).astype(np.float64)
    g_of_core = [[] for _ in range(NCORES)]
    for c in range(NCORES):
        gl = batch[n0[c]:n1[c]]
        if gl.size == 0:
            continue
        slots = gl - first_graph[c]
        assert slots.max() < G_SLOTS, "too many graphs on one core"
        loc = np.arange(counts[c])
        cs_pool[c, (loc // WIN) * 128 + (loc % WIN), slots] = \
            dis[n0[c]:n1[c]].astype(np.float16)
        for g in range(int(gl.min()), int(gl.max()) + 1):
            g_of_core[c].append(g)
            recip[c, g - first_graph[c], 0] = 1.0 / max(gcount[g], 1.0)

    # x fp16 padded [S_pad + XCHUNK*WIN + 1, H] per core (prologue transpose-
    # loads XCHUNK windows = XCHUNK*128 rows at a time)
    x_pad = np.zeros((NCORES, S_pad + XCHUNK * WIN + 1, H), dtype=np.float16)
    for c in range(NCORES):
        x_pad[c, :counts[c]] = x[n0[c]:n1[c]].astype(np.float16)

    host = dict(
        H=H, G=G, n_win=n_win, S_pad=S_pad, TBL_ROWS=TBL_ROWS, NBLK=NBLK,
        TOT_IDX=TOT_IDX, TOT_TILES=TOT_TILES, PK_TOT=PK_TOT,
        seg_off3=seg_off3, wtinst=wtinst,
        region=region, wgroups=wgroups, gtiles=gtiles, gbase=gbase,
        chunk_w0=chunk_w0, substep=substep, sub_rows=sub_rows,
        n0=n0, n1=n1, counts=counts, first_graph=first_graph,
        g_of_core=g_of_core,
    )
    # cs_pool device layout: [128, n_win*G_SLOTS], col block w -> window w
    cs_poolT = cs_pool.reshape(NCORES, n_win, 128, G_SLOTS) \
        .transpose(0, 2, 1, 3).reshape(NCORES, 128, n_win * G_SLOTS)
    per_core = [
        dict(
            xin=np.ascontiguousarray(x_pad[c]),
            idxs=np.ascontiguousarray(idx_wrapped[c]),
            starts=np.ascontiguousarray(starts_packed[c].reshape(1, -1)),
            dis_colt=np.ascontiguousarray(dis_pad[c].T),
            dis_colt2=np.ascontiguousarray((dis_pad[c] ** 2).T),
            cs_pool=np.ascontiguousarray(cs_poolT[c]),
            recip=np.ascontiguousarray(recip[c]),
        )
        for c in range(NCORES)
    ]
    return host, per_core


# --------------------------------------------------------------------------
# device program (shared across the 8 cores; per-core behavior is data-only)
# --------------------------------------------------------------------------

def _build_program(hp, L, single_core=False):
    H = hp["H"]
    n_win, S_pad = hp["n_win"], hp["S_pad"]
    TBL_ROWS, NBLK = hp["TBL_ROWS"], hp["NBLK"]
    TOT_IDX, TOT_TILES = hp["TOT_IDX"], hp["TOT_TILES"]
    PK_TOT = hp["PK_TOT"]
    seg_off3, wtinst = hp["seg_off3"], hp["wtinst"]

    nc = bacc.Bacc("TRN2", target_bir_lowering=False, debug=False,
                   num_devices=1 if single_core else NCORES,
                   num_swdge_queues=N_QUEUES)

    xin_d = nc.dram_tensor("xin", [S_pad + XCHUNK * WIN + 1, H], F16,
                           kind="ExternalInput")
    idx_d = nc.dram_tensor("idxs", [128, TOT_IDX // 16], I16, kind="ExternalInput")
    starts_d = nc.dram_tensor("starts", [1, PK_TOT], F16,
                              kind="ExternalInput")
    discolt_d = nc.dram_tensor("dis_colt", [128, n_win], F32, kind="ExternalInput")
    discolt2_d = nc.dram_tensor("dis_colt2", [128, n_win], F32, kind="ExternalInput")
    cspool_d = nc.dram_tensor("cs_pool", [128, n_win * G_SLOTS], F16, kind="ExternalInput")
    recip_d = nc.dram_tensor("recip", [G_SLOTS, 1], F32, kind="ExternalInput")
    iota_d = nc.dram_tensor("iota", [128, 1], F16, kind="ExternalInput")
    iotasig_d = nc.dram_tensor("iotasig", [128, 1], F32, kind="ExternalInput")
    ident_d = nc.dram_tensor("ident", [H, H], F16, kind="ExternalInput")
    w_d = nc.dram_tensor("w", [L * H, H], F16, kind="ExternalInput")
    out_d = nc.dram_tensor("out", [G_SLOTS, H], F32, kind="ExternalOutput")

    region = hp["region"]
    wgroups, gtiles, gbase = hp["wgroups"], hp["gtiles"], hp["gbase"]
    calls = []
    for b in range(NBLK):
        p = int(region[b, 0])
        while p < int(region[b, 1]):
            q = min(p + CHUNK, int(region[b, 1]))
            # first window consuming this chunk (for emission ordering)
            wf = int(np.searchsorted(seg_off3[:, b], p, side="right")) - 1
            calls.append((wf, b, p, q))
            p = q
    calls.sort()

    with tile.TileContext(nc) as tc:
        with tc.tile_pool(name="const", bufs=1) as cp, \
             tc.tile_pool(name="dram", bufs=1, space="DRAM") as dp, \
             tc.tile_pool(name="msg", bufs=MP_BUFS) as mp, \
             tc.tile_pool(name="sb", bufs=3) as sp, \
             tc.tile_pool(name="cs", bufs=CS_BUFS) as csp, \
             tc.tile_pool(name="ps", bufs=2, space="PSUM") as pp, \
             tc.tile_pool(name="suf_ps", bufs=3, space="PSUM") as pps, \
             tc.tile_pool(name="pool_ps", bufs=1, space="PSUM") as ppool:

            nc.gpsimd.load_library(_mlp_lib)

            iota_t = cp.tile([128, 1], F16)
            nc.sync.dma_start(iota_t[:], iota_d[:])
            iotasig_t = cp.tile([128, 1], F32)
            nc.sync.dma_start(iotasig_t[:], iotasig_d[:])
            ident_t = cp.tile([H, H], F16)
            nc.sync.dma_start(ident_t[:], ident_d[:])
            ones16 = cp.tile([1, 128], F16)
            nc.vector.memset(ones16[:], 1.0)
            w_tiles = []
            for l in range(L):
                wt = cp.tile([H, H], F16, tag=f"w{l}")
                nc.sync.dma_start(wt[:], w_d[l * H:(l + 1) * H, :])
                w_tiles.append(wt)
            recip_t = cp.tile([G_SLOTS, 1], F32)
            nc.sync.dma_start(recip_t[:], recip_d[:])
            discolt_t = cp.tile([128, n_win], F32)
            nc.sync.dma_start(discolt_t[:], discolt_d[:])
            discolt2_t = cp.tile([128, n_win], F32)
            nc.sync.dma_start(discolt2_t[:], discolt2_d[:])
            cspool_t = cp.tile([128, n_win * G_SLOTS], F16)
            nc.sync.dma_start(cspool_t[:], cspool_d[:])

            shard_a = dp.tile([S_pad, H], F16)
            shard_b = dp.tile([S_pad, H], F16)
            tspace = "Local" if single_core else "Shared"
            chunk_w0, substep = hp["chunk_w0"], hp["substep"]
            sub_rows = hp["sub_rows"]
            tables = [[dp.tile([sub_rows[k], H], F16, addr_space=tspace,
                               name=f"table{i}_{k}") for k in range(NBLK)]
                      for i in range(L)]

            def allgather_chunk(shard, table, k):
                lo = chunk_w0[k] * WIN
                if single_core:
                    nc.sync.dma_start(table[k][0:substep[k], :],
                                      shard[lo:lo + substep[k], :])
                else:
                    nc.gpsimd.collective_compute(
                        "AllGather", mybir.AluOpType.bypass,
                        replica_groups=[list(range(NCORES))],
                        ins=[shard[lo:lo + substep[k], :]],
                        outs=[table[k][:]])

            def transform(hT_f16, w_l, w, tq, qi, scale_t):
                """t[127,H] = scale * (h @ W) into group tile column qi."""
                tps = pp.tile([WIN, H], F32, space="PSUM", tag="tps")
                nc.tensor.matmul(out=tps[:], lhsT=hT_f16[:, :WIN], rhs=w_l[:],
                                 start=True, stop=True)
                nc.vector.tensor_scalar_mul(tq[:, qi, :], tps[:],
                                            scale_t[0:WIN, w:w + 1])

            def flush_group(tq, grp, shard):
                w0, glen = grp[0], len(grp)
                dst = shard[w0 * WIN:(w0 + glen) * WIN, :].rearrange(
                    "(q p) h -> p q h", p=WIN)
                nc.sync.dma_start(dst, tq[:, :glen, :])

            GMAX = max(len(g) for g in wgroups)

            # ---------------- layer 0 prologue: t0 = dis*(x@W0) --------------
            # transposed x loads, XCHUNK windows per DMA
            xT_of = {}
            for w0 in range(0, n_win, XCHUNK):
                nw = min(XCHUNK, n_win - w0)
                xT = sp.tile([H, XCHUNK * 128], F16, tag="xT")
                nc.sync.dma_start(xT[:, :nw * 128],
                                  xin_d[w0 * WIN:w0 * WIN + nw * 128, :],
                                  transpose=True)
                for k in range(nw):
                    xT_of[w0 + k] = xT[:, k * WIN:k * WIN + 128]
            for grp in wgroups:
                tq = sp.tile([WIN, GMAX, H], F16, tag="t_sb")
                for qi, w in enumerate(grp):
                    transform(xT_of[w], w_tiles[0], w, tq, qi, discolt_t)
                flush_group(tq, grp, shard_a)
                wend = grp[-1] + 1
                if wend in chunk_w0:
                    allgather_chunk(shard_a, tables[0],
                                    chunk_w0.index(wend) - 1)

            # ---------------- layers ----------------
            pool_ps = ppool.tile([G_SLOTS, H], F32, space="PSUM")
            n_pool_mm = 0

            for l in range(L):
                last = l == L - 1
                table = tables[l]
                nshard = shard_b if l % 2 == 0 else shard_a
                ntable = tables[l + 1] if not last else None

                chunk_tiles = []
                for ci, (wf, b, lo, hi) in enumerate(calls):
                    n = hi - lo
                    it = sp.tile([128, n // 16], I16, tag=f"idx{b}")
                    nc.sync.dma_start(it[:], idx_d[:, lo // 16:hi // 16])
                    mt = mp.tile([128, n // 128, H], F16, tag=f"m{b}")
                    nc.gpsimd.dma_gather(
                        mt[:], table[b][:],
                        it[:], n, n, H, single_packet=False,
                        queue_num=ci % N_QUEUES)
                    chunk_tiles.append((lo, hi, mt))

                def m_tile(p):
                    for lo, hi, mt in chunk_tiles:
                        if lo <= p < hi:
                            return mt[:, (p - lo) // 128, :]
                    raise AssertionError(p)

                csalt_box = [0]

                def build_cs(gi):
                    """stg DMA + bps broadcast + staircase compare for all
                    4-tile chunks of group gi; returns cs_of."""
                    tl = gtiles[gi]
                    cs_of = {}
                    if not tl:
                        return cs_of
                    nchunk = (len(tl) + 3) // 4
                    stg = sp.tile([1, 8192], F16, tag="starts", bufs=2)
                    nc.sync.dma_start(
                        stg[:, :nchunk * 512],
                        starts_d[:, gbase[gi]:gbase[gi] + nchunk * 512])
                    for gs in range(0, len(tl), 4):
                        gl = min(4, len(tl) - gs)
                        gn = gl * 128
                        bps = pp.tile([128, 512], F32, space="PSUM",
                                      tag="bps")
                        nc.tensor.matmul(
                            out=bps[:, :gn], lhsT=ones16[:],
                            rhs=stg[:, gs * 128:gs * 128 + gn],
                            start=True, stop=True)
                        cs = csp.tile([128, 512], F16, tag="cs")
                        if csalt_box[0] % 2 == 0:
                            nc.vector.tensor_tensor(
                                out=cs[:, :gn],
                                in0=iota_t[:].to_broadcast([128, gn]),
                                in1=bps[:, :gn],
                                op=mybir.AluOpType.is_ge)
                        else:
                            nc.scalar.activation(
                                out=cs[:, :gn], in_=bps[:, :gn],
                                func=mybir.ActivationFunctionType.Sigmoid,
                                bias=iotasig_t[:], scale=-64.0)
                        csalt_box[0] += 1
                        for k in range(gl):
                            cs_of[tl[gs + k]] = cs[:, k * 128:(k + 1) * 128]
                    return cs_of

                cs_cur = build_cs(0)
                for gi, grp in enumerate(wgroups):
                    cs_of = cs_cur
                    tq = None
                    if not last:
                        tq = sp.tile([WIN, GMAX, H], F16, tag="t_sb")

                    # ---- phase B: suffix matmuls + relu per window ----
                    hT_of = {}
                    for qi, w in enumerate(grp):
                        inst_w = wtinst[w]
                        ntw = len(inst_w)
                        a_t = sp.tile([H, WIN], F32, tag="a")
                        if ntw:
                            suf = pps.tile([H, 128], F32, space="PSUM",
                                           tag="suf")
                            for k, (p, b, beta) in enumerate(inst_w):
                                nc.tensor.matmul(out=suf[:], lhsT=m_tile(p),
                                                 rhs=cs_of[(w, p)],
                                                 start=(k == 0),
                                                 stop=(k == ntw - 1))
                            # a = suf[:, :127] - suf[:, 1:128]; one copy to
                            # SBUF (tensor_tensor can't read PSUM twice)
                            suf_sb = sp.tile([H, WIN], F32, tag="suf_sb")
                            nc.vector.tensor_copy(suf_sb[:],
                                                  suf[:, 1:WIN + 1])
                            nc.vector.tensor_tensor(out=a_t[:],
                                                    in0=suf[:, :WIN],
                                                    in1=suf_sb[:],
                                                    op=mybir.AluOpType.subtract)
                        else:
                            nc.vector.memset(a_t[:], 0.0)

                        hT = sp.tile([H, WIN], F16, tag="hT", bufs=10)
                        nc.scalar.activation(
                            out=hT[:], in_=a_t[:],
                            func=mybir.ActivationFunctionType.Relu,
                            scale=1.0)
                        hT_of[w] = hT

                    # ---- build next group's cs while transforms drain ----
                    if gi + 1 < len(wgroups):
                        cs_cur = build_cs(gi + 1)

                    # ---- phase C: transforms / pooling ----
                    for qi, w in enumerate(grp):
                        hT = hT_of[w]
                        if not last:
                            transform(hT, w_tiles[l + 1], w, tq, qi,
                                      discolt2_t)
                        else:
                            # pooling: node-major h then pool_ps += csw^T h
                            # (dst-side dis is folded into cspool)
                            hnm_ps = pp.tile([WIN, H], F32, space="PSUM",
                                             tag="tps")
                            nc.tensor.matmul(out=hnm_ps[:], lhsT=hT[:, :WIN],
                                             rhs=ident_t[:], start=True,
                                             stop=True)
                            hnm = sp.tile([WIN, H], F16, tag="hnm")
                            nc.vector.tensor_copy(hnm[:], hnm_ps[:])
                            nc.tensor.matmul(
                                out=pool_ps[:],
                                lhsT=cspool_t[0:WIN,
                                              w * G_SLOTS:(w + 1) * G_SLOTS],
                                rhs=hnm[:],
                                start=(n_pool_mm == 0),
                                stop=(n_pool_mm == n_win - 1))
                            n_pool_mm += 1
                    if not last:
                        flush_group(tq, grp, nshard)
                        # fire the AllGather chunk as soon as its last
                        # group is flushed (pipelines with later groups)
                        wend = grp[-1] + 1
                        if wend in chunk_w0:
                            k = chunk_w0.index(wend) - 1
                            allgather_chunk(nshard, ntable, k)

            pool_sb = sp.tile([G_SLOTS, H], F32, tag="pool_sb")
            nc.vector.tensor_scalar_mul(pool_sb[:], pool_ps[:], recip_t[:])
            nc.sync.dma_start(out_d[:], pool_sb[:])

    nc.compile()
    return nc


# --------------------------------------------------------------------------

def kernel(x, edge_index, batch, Ws, bs):
    x = np.asarray(x)
    edge_index = np.asarray(edge_index)
    batch = np.asarray(batch)
    Ws = np.asarray(Ws, dtype=np.float32)
    bs = np.asarray(bs, dtype=np.float32)
    assert np.all(bs == 0.0), "kernel folds deg-scale through relu; needs b=0"
    L, H = Ws.shape[0], Ws.shape[1]

    host, per_core = _build_host(x, edge_index, batch)
    nc = _build_program(host, L)

    iota_col = np.arange(128, dtype=np.float16).reshape(128, 1)
    iotasig = (64.0 * np.arange(128) + 32.0).astype(np.float32).reshape(128, 1)
    ident = np.eye(H, dtype=np.float16)
    w_fp16 = np.ascontiguousarray(Ws.astype(np.float16).reshape(L * H, H))
    in_maps = [
        dict(pc, iota=iota_col, iotasig=iotasig, ident=ident, w=w_fp16)
        for pc in per_core
    ]
    res = run_bass_kernel_spmd(nc, in_maps, core_ids=list(range(NCORES)))

    G = host["G"]
    out = np.zeros((G, H), dtype=np.float32)
    for c in range(NCORES):
        fg = int(host["first_graph"][c])
        for g in host["g_of_core"][c]:
            out[g] = res.results[c]["out"][g - fg]
    return out

